# revision 21
# baseline (speedup 1.0000x reference)
"""Trainium2 kernel for nn_NormalizedHungarianLoss.

Semantics (mirrors the reference): per-sample min-max normalize the cost
matrix in float64 on the host, solve the Hungarian assignment on the host
(the reference itself does this on CPU via numpy), then compute the loss
mean(D[b, r, cols[b, r]]) from the ORIGINAL float32 D on device.

Device part (data-parallel over batch, 4 samples per core):
  - The core's [2048, 512] f32 shard is viewed as [128, 8192] (partition p
    holds the 16 consecutive rows 16p..16p+15), giving 16 KiB contiguous
    DMA descriptors per partition; 2 dma_starts of 2 MiB each.
  - One GPSIMD indirect_copy per tile gathers candidate matched elements.
    indirect_copy shares the index list across each 16-partition group
    (out[p, i] = data[p, idx_i]); index i = k*16 + j carries the matched
    element of row 16*(lo+k) + t*8 + j, valid only at partition p = lo+k.
  - DVE multiply by a constant 0/1 mask selecting k == p%16, reduce over
    the free dim -> [128, 1] partials, then a GPSIMD partition-reduce to a
    single [1, 1] scalar (so the output DMA is one descriptor).
Host sums the 8 per-core scalars and divides by B*N.
"""

import sys

import numpy as np

if "/opt/trn_rl_repo" not in sys.path:
    sys.path.insert(0, "/opt/trn_rl_repo")

_B, _N = 32, 512
_NCORES = 8
_BPC = _B // _NCORES            # samples per core
_ROWS = _BPC * _N               # 2048 rows per core
_RPP = _ROWS // 128             # 16 rows per partition
_NT = 1                         # DMA tiles per core
_RPT = _RPP // _NT              # 8 rows per partition per tile


# ---------------------------------------------------------------------------
# Host-side Hungarian matching (same math as the reference, CPU/numpy).
# ---------------------------------------------------------------------------

def _lap(cost):
    """Jonker-Volgenant O(n^3) min-cost assignment on a square float64
    matrix; returns the column assigned to each row."""
    n = cost.shape[0]
    INF = np.inf
    C = np.zeros((n + 1, n + 1), dtype=np.float64)
    C[1:, 1:] = cost
    u = np.zeros(n + 1)
    v = np.zeros(n + 1)
    p = np.zeros(n + 1, dtype=np.int64)
    way = np.zeros(n + 1, dtype=np.int64)
    for i in range(1, n + 1):
        p[0] = i
        j0 = 0
        minv = np.full(n + 1, INF)
        used = np.zeros(n + 1, dtype=bool)
        while True:
            used[j0] = True
            i0 = p[j0]
            free = ~used
            free[0] = False
            cur = C[i0] - u[i0] - v
            upd = free & (cur < minv)
            minv = np.where(upd, cur, minv)
            way[upd] = j0
            mv = np.where(free, minv, INF)
            j1 = int(np.argmin(mv))
            delta = mv[j1]
            u[p[used]] += delta
            v[used] -= delta
            minv = np.where(free, minv - delta, minv)
            j0 = j1
            if p[j0] == 0:
                break
        while j0 != 0:
            j1 = way[j0]
            p[j0] = p[j1]
            j0 = j1
    col_for_row = np.zeros(n, dtype=np.int64)
    col_for_row[p[1:] - 1] = np.arange(n)
    return col_for_row


def _match(D_np):
    """Per-sample min-max normalize (float64) then Hungarian match."""
    B, N, _ = D_np.shape
    cols = np.zeros((B, N), dtype=np.int64)
    try:
        from scipy.optimize import linear_sum_assignment
    except Exception:
        linear_sum_assignment = None
    for b in range(B):
        Db = D_np[b]
        d_min = Db.min()
        d_range = Db.max() - d_min + 1e-8
        C = (Db - d_min) / d_range
        if linear_sum_assignment is not None:
            _, cols[b] = linear_sum_assignment(C)
        else:
            cols[b] = _lap(C)
    return cols


# ---------------------------------------------------------------------------
# Device kernel.
# ---------------------------------------------------------------------------

_NC_CACHE = None
_NC_RAW_CACHE = None


def _build_nc_raw():
    """Raw-Bass (no Tile) variant: explicit semaphores, no tail butterfly."""
    global _NC_RAW_CACHE
    if _NC_RAW_CACHE is not None:
        return _NC_RAW_CACHE

    import concourse.bass as bass
    from concourse import mybir

    nc = bass.Bass(target_bir_lowering=False)

    d_in = nc.dram_tensor("d", [_ROWS, _N], mybir.dt.float32, kind="ExternalInput")
    idx_in = nc.dram_tensor("idx", [128, _RPP], mybir.dt.uint16,
                            kind="ExternalInput")
    mask_in = nc.dram_tensor("mask", [128, _RPP * 16], mybir.dt.float32,
                             kind="ExternalInput")
    out = nc.dram_tensor("out", [1, 1], mybir.dt.float32, kind="ExternalOutput")

    # partition p holds rows 16p..16p+15 contiguously -> 16 KiB descriptors
    d_view = d_in.ap().rearrange("(p j) m -> p (j m)", p=128)  # [128, 8192]

    idx_t = nc.alloc_sbuf_tensor("idx_t", [128, _RPP], mybir.dt.uint16)
    mask_t = nc.alloc_sbuf_tensor("mask_t", [128, _RPP * 16], mybir.dt.float32)
    gath = nc.alloc_sbuf_tensor("gath", [128, _RPP * 16], mybir.dt.float32)
    prod = nc.alloc_sbuf_tensor("prod", [128, _RPP * 16], mybir.dt.float32)
    part = nc.alloc_sbuf_tensor("part", [128, 1], mybir.dt.float32)
    sc = nc.alloc_sbuf_tensor("sc", [1, 1], mybir.dt.float32)
    data = [nc.alloc_sbuf_tensor(f"data{t}", [128, _RPT * _N], mybir.dt.float32)
            for t in range(_NT)]

    idx_sem = nc.alloc_semaphore("idx_sem")
    mask_sem = nc.alloc_semaphore("mask_sem")
    d_sems = [nc.alloc_semaphore(f"d{t}_sem") for t in range(_NT)]
    out_sem = nc.alloc_semaphore("out_sem")
    gsem = nc.alloc_semaphore("gsem")
    vsem = nc.alloc_semaphore("vsem")
    all_sems = [idx_sem, mask_sem] + d_sems + [out_sem, gsem, vsem]

    with nc.Block() as block:

        @block.sync
        def _(sync):
            sync.dma_start(idx_t.ap(), idx_in.ap()).then_inc(idx_sem, 16)
            for t in range(_NT):
                sync.dma_start(
                    data[t].ap(),
                    d_view[:, t * _RPT * _N:(t + 1) * _RPT * _N],
                ).then_inc(d_sems[t], 16)
            sync.dma_start(mask_t.ap(), mask_in.ap()).then_inc(mask_sem, 16)
            sync.wait_ge(vsem, 3)
            sync.dma_start(out.ap(), sc.ap()).then_inc(out_sem, 16)
            sync.wait_ge(out_sem, 16)

        @block.gpsimd
        def _(gpsimd):
            gpsimd.wait_ge(idx_sem, 16)
            for t in range(_NT):
                gpsimd.wait_ge(d_sems[t], 16)
                nc.gpsimd.indirect_copy(
                    gath.ap()[:, t * _RPT * 16:(t + 1) * _RPT * 16],
                    data[t].ap(),
                    idx_t.ap()[:, t * _RPT:(t + 1) * _RPT],
                    True,
                ).then_inc(gsem, 1)
            gpsimd.wait_ge(vsem, 2)
            nc.gpsimd.tensor_reduce(sc.ap(), part.ap(), mybir.AxisListType.C,
                                    mybir.AluOpType.add).then_inc(vsem, 1)

        @block.vector
        def _(vector):
            vector.wait_ge(gsem, _NT)
            vector.wait_ge(mask_sem, 16)
            nc.vector.tensor_mul(prod.ap(), gath.ap(), mask_t.ap()).then_inc(vsem, 1)
            vector.wait_ge(vsem, 1)
            nc.vector.tensor_reduce(part.ap(), prod.ap(), mybir.AxisListType.X,
                                    mybir.AluOpType.add).then_inc(vsem, 1)

    # Block exit emitted drain + all-engine barrier; now restore semaphores
    # to 0 so the NEFF can re-execute.
    nums = sorted(s.num for s in all_sems)
    spans = [[nums[0], nums[0]]]
    for n in nums[1:]:
        if n == spans[-1][1] + 1:
            spans[-1][1] = n
        else:
            spans.append([n, n])
    for lo, hi in spans:
        nc.gpsimd.dma_reset(range(lo, hi + 1))
        nc.gpsimd.sem_clear(range(lo, hi + 1))

    _NC_RAW_CACHE = nc
    return nc


def _build_nc():
    global _NC_CACHE
    if _NC_CACHE is not None:
        return _NC_CACHE

    import concourse.bacc as bacc
    import concourse.tile as tile
    from concourse import mybir

    nc = bacc.Bacc("TRN2", target_bir_lowering=False, debug=False)

    d_in = nc.dram_tensor("d", [_ROWS, _N], mybir.dt.float32, kind="ExternalInput")
    idx_in = nc.dram_tensor("idx", [128, _RPP], mybir.dt.uint16,
                            kind="ExternalInput")
    mask_in = nc.dram_tensor("mask", [128, _RPP * 16], mybir.dt.float32,
                             kind="ExternalInput")
    out = nc.dram_tensor("out", [1, 1], mybir.dt.float32, kind="ExternalOutput")

    d_view = d_in.ap().rearrange("(p j) m -> p (j m)", p=128)  # [128, 8192]

    with tile.TileContext(nc) as tc:
        with tc.tile_pool(name="const", bufs=1) as cpool, \
             tc.tile_pool(name="data", bufs=_NT) as dpool:
            idx_t = cpool.tile([128, _RPP], mybir.dt.uint16)
            nc.sync.dma_start(idx_t[:], idx_in.ap())
            mask_t = cpool.tile([128, _RPP * 16], mybir.dt.float32)
            nc.sync.dma_start(mask_t[:], mask_in.ap())

            gath = cpool.tile([128, _RPP * 16], mybir.dt.float32)
            for t in range(_NT):
                dt_ = dpool.tile([128, _RPT * _N], mybir.dt.float32)
                nc.sync.dma_start(
                    dt_[:],
                    d_view[:, t * _RPT * _N:(t + 1) * _RPT * _N],
                )
                nc.gpsimd.indirect_copy(
                    gath[:, t * _RPT * 16:(t + 1) * _RPT * 16],
                    dt_[:],
                    idx_t[:, t * _RPT:(t + 1) * _RPT],
                    True,
                )

            prod = cpool.tile([128, _RPP * 16], mybir.dt.float32)
            part = cpool.tile([128, 1], mybir.dt.float32)
            sc = cpool.tile([1, 1], mybir.dt.float32)
            nc.vector.tensor_mul(prod[:], gath[:], mask_t[:])
            nc.vector.tensor_reduce(part[:], prod[:], mybir.AxisListType.X,
                                    mybir.AluOpType.add)
            nc.gpsimd.tensor_reduce(sc[:], part[:], mybir.AxisListType.C,
                                    mybir.AluOpType.add)
            nc.sync.dma_start(out.ap(), sc[:])

    nc.compile()
    _NC_CACHE = nc
    return nc


def _core_inputs(D_np, cols_flat):
    """Per-core input maps for run_bass_kernel_spmd."""
    # gather slot i carries partition-group member k = i % 16's value
    ii = np.arange(_RPP * 16)
    pp = np.arange(128)
    mask = ((ii[None, :] % 16) == (pp[:, None] % 16)).astype(np.float32)

    in_maps = []
    for c in range(_NCORES):
        d_shard = np.ascontiguousarray(
            D_np[c * _BPC:(c + 1) * _BPC].reshape(_ROWS, _N))
        cshard = cols_flat[c * _ROWS:(c + 1) * _ROWS]
        # idx[q, t*_RPT + j] = j*N + cols[16q + t*_RPT + j]
        cj = cshard.reshape(128, _RPP)          # [q, u] with u = t*_RPT + j
        j = np.arange(_RPP) % _RPT
        idx = (j[None, :] * _N + cj).astype(np.uint16)
        in_maps.append({"d": d_shard, "idx": idx, "mask": mask})
    return in_maps


_USE_RAW = True


def _run_device(D_np, cols_flat, trace=False):
    from concourse.bass_utils import run_bass_kernel_spmd

    nc = _build_nc_raw() if _USE_RAW else _build_nc()
    in_maps = _core_inputs(D_np, cols_flat)
    res = run_bass_kernel_spmd(nc, in_maps, list(range(_NCORES)), trace=trace)
    parts = np.stack([r["out"] for r in res.results])  # [8, 1, 1]
    return parts, res


def kernel(D):
    D_np = np.asarray(D, dtype=np.float32)
    cols = _match(D_np.astype(np.float64))              # [B, N] host, like ref
    cols_flat = cols.reshape(-1).astype(np.int64)
    parts, _ = _run_device(D_np, cols_flat)
    total = parts.astype(np.float64).sum()
    return np.asarray(total / (_B * _N), dtype=np.float32)


# revision 28
# speedup vs baseline: 1.0440x; 1.0440x over previous
"""Trainium2 kernel for nn_NormalizedHungarianLoss.

Semantics (mirrors the reference): per-sample min-max normalize the cost
matrix in float64 on the host, solve the Hungarian assignment on the host
(the reference itself does this on CPU via numpy), then compute the loss
mean(D[b, r, cols[b, r]]) from the ORIGINAL float32 D on device.

Device part (data-parallel over batch, 4 samples per core):
  - The core's [2048, 512] f32 shard is viewed as [128, 8192] (partition p
    holds the 16 consecutive rows 16p..16p+15), giving 16 KiB contiguous
    DMA descriptors per partition; 2 dma_starts of 2 MiB each.
  - One GPSIMD indirect_copy per tile gathers candidate matched elements.
    indirect_copy shares the index list across each 16-partition group
    (out[p, i] = data[p, idx_i]); index i = k*16 + j carries the matched
    element of row 16*(lo+k) + t*8 + j, valid only at partition p = lo+k.
  - DVE multiply by a constant 0/1 mask selecting k == p%16, reduce over
    the free dim -> [128, 1] partials, then a GPSIMD partition-reduce to a
    single [1, 1] scalar (so the output DMA is one descriptor).
Host sums the 8 per-core scalars and divides by B*N.
"""

import sys

import numpy as np

if "/opt/trn_rl_repo" not in sys.path:
    sys.path.insert(0, "/opt/trn_rl_repo")

_B, _N = 32, 512
_NCORES = 8
_BPC = _B // _NCORES            # samples per core
_ROWS = _BPC * _N               # 2048 rows per core
_RPP = _ROWS // 128             # 16 rows per partition
_NT = 1                         # DMA tiles per core
_RPT = _RPP // _NT              # 8 rows per partition per tile


# ---------------------------------------------------------------------------
# Host-side Hungarian matching (same math as the reference, CPU/numpy).
# ---------------------------------------------------------------------------

def _lap(cost):
    """Jonker-Volgenant O(n^3) min-cost assignment on a square float64
    matrix; returns the column assigned to each row."""
    n = cost.shape[0]
    INF = np.inf
    C = np.zeros((n + 1, n + 1), dtype=np.float64)
    C[1:, 1:] = cost
    u = np.zeros(n + 1)
    v = np.zeros(n + 1)
    p = np.zeros(n + 1, dtype=np.int64)
    way = np.zeros(n + 1, dtype=np.int64)
    for i in range(1, n + 1):
        p[0] = i
        j0 = 0
        minv = np.full(n + 1, INF)
        used = np.zeros(n + 1, dtype=bool)
        while True:
            used[j0] = True
            i0 = p[j0]
            free = ~used
            free[0] = False
            cur = C[i0] - u[i0] - v
            upd = free & (cur < minv)
            minv = np.where(upd, cur, minv)
            way[upd] = j0
            mv = np.where(free, minv, INF)
            j1 = int(np.argmin(mv))
            delta = mv[j1]
            u[p[used]] += delta
            v[used] -= delta
            minv = np.where(free, minv - delta, minv)
            j0 = j1
            if p[j0] == 0:
                break
        while j0 != 0:
            j1 = way[j0]
            p[j0] = p[j1]
            j0 = j1
    col_for_row = np.zeros(n, dtype=np.int64)
    col_for_row[p[1:] - 1] = np.arange(n)
    return col_for_row


def _match(D_np):
    """Per-sample min-max normalize (float64) then Hungarian match."""
    B, N, _ = D_np.shape
    cols = np.zeros((B, N), dtype=np.int64)
    try:
        from scipy.optimize import linear_sum_assignment
    except Exception:
        linear_sum_assignment = None
    for b in range(B):
        Db = D_np[b]
        d_min = Db.min()
        d_range = Db.max() - d_min + 1e-8
        C = (Db - d_min) / d_range
        if linear_sum_assignment is not None:
            _, cols[b] = linear_sum_assignment(C)
        else:
            cols[b] = _lap(C)
    return cols


# ---------------------------------------------------------------------------
# Device kernel.
# ---------------------------------------------------------------------------

_NC_CACHE = None
_NC_RAW_CACHE = None


def _build_nc_raw():
    """Raw-Bass (no Tile) variant: explicit semaphores, no tail butterfly."""
    global _NC_RAW_CACHE
    if _NC_RAW_CACHE is not None:
        return _NC_RAW_CACHE

    import concourse.bass as bass
    from concourse import mybir

    nc = bass.Bass(target_bir_lowering=False)

    d_in = nc.dram_tensor("d", [_ROWS, _N], mybir.dt.float32, kind="ExternalInput")
    # last idx column is zeros, used by the warm-up gather
    idx_in = nc.dram_tensor("idx", [128, _RPP + 1], mybir.dt.uint16,
                            kind="ExternalInput")
    mask_in = nc.dram_tensor("mask", [128, _RPP * 16], mybir.dt.float32,
                             kind="ExternalInput")
    out = nc.dram_tensor("out", [1, 1], mybir.dt.float32, kind="ExternalOutput")

    # partition p holds rows 16p..16p+15 contiguously -> 32 KiB descriptors
    d_view = d_in.ap().rearrange("(p j) m -> p (j m)", p=128)  # [128, 8192]

    idx_t = nc.alloc_sbuf_tensor("idx_t", [128, _RPP + 1], mybir.dt.uint16)
    mask_t = nc.alloc_sbuf_tensor("mask_t", [128, _RPP * 16], mybir.dt.float32)
    gath = nc.alloc_sbuf_tensor("gath", [128, _RPP * 16], mybir.dt.float32)
    prod = nc.alloc_sbuf_tensor("prod", [128, _RPP * 16], mybir.dt.float32)
    part = nc.alloc_sbuf_tensor("part", [128, 1], mybir.dt.float32)
    sc = nc.alloc_sbuf_tensor("sc", [1, 1], mybir.dt.float32)
    junk = nc.alloc_sbuf_tensor("junk", [128, 16], mybir.dt.float32)
    wdata = nc.alloc_sbuf_tensor("wdata", [128, 16], mybir.dt.float32)
    data = [nc.alloc_sbuf_tensor(f"data{t}", [128, _RPT * _N], mybir.dt.float32)
            for t in range(_NT)]

    idx_sem = nc.alloc_semaphore("idx_sem")
    mask_sem = nc.alloc_semaphore("mask_sem")
    d_sems = [nc.alloc_semaphore(f"d{t}_sem") for t in range(_NT)]
    out_sem = nc.alloc_semaphore("out_sem")
    gsem = nc.alloc_semaphore("gsem")
    vsem = nc.alloc_semaphore("vsem")
    all_sems = [idx_sem, mask_sem] + d_sems + [out_sem, gsem, vsem]

    with nc.Block() as block:

        @block.sync
        def _(sync):
            for t in range(_NT):
                sync.dma_start(
                    data[t].ap(),
                    d_view[:, t * _RPT * _N:(t + 1) * _RPT * _N],
                ).then_inc(d_sems[t], 16)
            sync.dma_start(idx_t.ap(), idx_in.ap()).then_inc(idx_sem, 16)
            sync.dma_start(mask_t.ap(), mask_in.ap()).then_inc(mask_sem, 16)
            sync.wait_ge(vsem, 3)
            sync.dma_start(out.ap(), sc.ap()).then_inc(out_sem, 16)
            sync.wait_ge(out_sem, 16)

        @block.gpsimd
        def _(gpsimd):
            gpsimd.wait_ge(idx_sem, 16)
            # warm-up: loads the Q7 IndirectCopy ucode while the data DMA
            # is still in flight (zero indices gather junk from idx_t)
            nc.gpsimd.indirect_copy(
                junk.ap(),
                wdata.ap(),
                idx_t.ap()[:, _RPP:_RPP + 1],
                True,
            )
            for t in range(_NT):
                gpsimd.wait_ge(d_sems[t], 16)
                nc.gpsimd.indirect_copy(
                    gath.ap()[:, t * _RPT * 16:(t + 1) * _RPT * 16],
                    data[t].ap(),
                    idx_t.ap()[:, t * _RPT:(t + 1) * _RPT],
                    True,
                ).then_inc(gsem, 1)
            gpsimd.wait_ge(vsem, 2)
            nc.gpsimd.tensor_reduce(sc.ap(), part.ap(), mybir.AxisListType.C,
                                    mybir.AluOpType.add).then_inc(vsem, 1)

        @block.vector
        def _(vector):
            vector.wait_ge(gsem, _NT)
            vector.wait_ge(mask_sem, 16)
            nc.vector.tensor_mul(prod.ap(), gath.ap(), mask_t.ap()).then_inc(vsem, 1)
            vector.wait_ge(vsem, 1)
            nc.vector.tensor_reduce(part.ap(), prod.ap(), mybir.AxisListType.X,
                                    mybir.AluOpType.add).then_inc(vsem, 1)

    # Block exit emitted drain + all-engine barrier; now restore semaphores
    # to 0 so the NEFF can re-execute.
    nums = sorted(s.num for s in all_sems)
    spans = [[nums[0], nums[0]]]
    for n in nums[1:]:
        if n == spans[-1][1] + 1:
            spans[-1][1] = n
        else:
            spans.append([n, n])
    for lo, hi in spans:
        nc.gpsimd.dma_reset(range(lo, hi + 1))
        nc.gpsimd.sem_clear(range(lo, hi + 1))

    _NC_RAW_CACHE = nc
    return nc


def _build_nc():
    global _NC_CACHE
    if _NC_CACHE is not None:
        return _NC_CACHE

    import concourse.bacc as bacc
    import concourse.tile as tile
    from concourse import mybir

    nc = bacc.Bacc("TRN2", target_bir_lowering=False, debug=False)

    d_in = nc.dram_tensor("d", [_ROWS, _N], mybir.dt.float32, kind="ExternalInput")
    idx_in = nc.dram_tensor("idx", [128, _RPP + 1], mybir.dt.uint16,
                            kind="ExternalInput")
    mask_in = nc.dram_tensor("mask", [128, _RPP * 16], mybir.dt.float32,
                             kind="ExternalInput")
    out = nc.dram_tensor("out", [1, 1], mybir.dt.float32, kind="ExternalOutput")

    d_view = d_in.ap().rearrange("(p j) m -> p (j m)", p=128)  # [128, 8192]

    with tile.TileContext(nc) as tc:
        with tc.tile_pool(name="const", bufs=1) as cpool, \
             tc.tile_pool(name="data", bufs=_NT) as dpool:
            idx_t = cpool.tile([128, _RPP + 1], mybir.dt.uint16)
            nc.sync.dma_start(idx_t[:], idx_in.ap())
            mask_t = cpool.tile([128, _RPP * 16], mybir.dt.float32)
            nc.sync.dma_start(mask_t[:], mask_in.ap())

            gath = cpool.tile([128, _RPP * 16], mybir.dt.float32)
            for t in range(_NT):
                dt_ = dpool.tile([128, _RPT * _N], mybir.dt.float32)
                nc.sync.dma_start(
                    dt_[:],
                    d_view[:, t * _RPT * _N:(t + 1) * _RPT * _N],
                )
                nc.gpsimd.indirect_copy(
                    gath[:, t * _RPT * 16:(t + 1) * _RPT * 16],
                    dt_[:],
                    idx_t[:, t * _RPT:(t + 1) * _RPT],
                    True,
                )

            prod = cpool.tile([128, _RPP * 16], mybir.dt.float32)
            part = cpool.tile([128, 1], mybir.dt.float32)
            sc = cpool.tile([1, 1], mybir.dt.float32)
            nc.vector.tensor_mul(prod[:], gath[:], mask_t[:])
            nc.vector.tensor_reduce(part[:], prod[:], mybir.AxisListType.X,
                                    mybir.AluOpType.add)
            nc.gpsimd.tensor_reduce(sc[:], part[:], mybir.AxisListType.C,
                                    mybir.AluOpType.add)
            nc.sync.dma_start(out.ap(), sc[:])

    nc.compile()
    _NC_CACHE = nc
    return nc


def _core_inputs(D_np, cols_flat):
    """Per-core input maps for run_bass_kernel_spmd."""
    # gather slot i carries partition-group member k = i % 16's value
    ii = np.arange(_RPP * 16)
    pp = np.arange(128)
    mask = ((ii[None, :] % 16) == (pp[:, None] % 16)).astype(np.float32)

    in_maps = []
    for c in range(_NCORES):
        d_shard = np.ascontiguousarray(
            D_np[c * _BPC:(c + 1) * _BPC].reshape(_ROWS, _N))
        cshard = cols_flat[c * _ROWS:(c + 1) * _ROWS]
        # idx[q, t*_RPT + j] = j*N + cols[16q + t*_RPT + j]; last col zeros
        cj = cshard.reshape(128, _RPP)          # [q, u] with u = t*_RPT + j
        j = np.arange(_RPP) % _RPT
        idx = np.zeros((128, _RPP + 1), dtype=np.uint16)
        idx[:, :_RPP] = j[None, :] * _N + cj
        in_maps.append({"d": d_shard, "idx": idx, "mask": mask})
    return in_maps


_USE_RAW = True


def _run_device(D_np, cols_flat, trace=False):
    from concourse.bass_utils import run_bass_kernel_spmd

    nc = _build_nc_raw() if _USE_RAW else _build_nc()
    in_maps = _core_inputs(D_np, cols_flat)
    res = run_bass_kernel_spmd(nc, in_maps, list(range(_NCORES)), trace=trace)
    parts = np.stack([r["out"] for r in res.results])  # [8, 1, 1]
    return parts, res


def kernel(D):
    D_np = np.asarray(D, dtype=np.float32)
    cols = _match(D_np.astype(np.float64))              # [B, N] host, like ref
    cols_flat = cols.reshape(-1).astype(np.int64)
    parts, _ = _run_device(D_np, cols_flat)
    total = parts.astype(np.float64).sum()
    return np.asarray(total / (_B * _N), dtype=np.float32)


# revision 33
# speedup vs baseline: 1.1176x; 1.0704x over previous
"""Trainium2 kernel for nn_NormalizedHungarianLoss.

Semantics (mirrors the reference): per-sample min-max normalize the cost
matrix in float64 on the host, solve the Hungarian assignment on the host
(the reference itself does this on CPU via numpy), then compute the loss
mean(D[b, r, cols[b, r]]) from the ORIGINAL float32 D on device.

Device part (data-parallel over batch, 4 samples per core):
  - The core's [2048, 512] f32 shard is viewed as [128, 8192] (partition p
    holds the 16 consecutive rows 16p..16p+15), giving 16 KiB contiguous
    DMA descriptors per partition; 2 dma_starts of 2 MiB each.
  - One GPSIMD indirect_copy per tile gathers candidate matched elements.
    indirect_copy shares the index list across each 16-partition group
    (out[p, i] = data[p, idx_i]); index i = k*16 + j carries the matched
    element of row 16*(lo+k) + t*8 + j, valid only at partition p = lo+k.
  - DVE multiply by a constant 0/1 mask selecting k == p%16, reduce over
    the free dim -> [128, 1] partials, then a GPSIMD partition-reduce to a
    single [1, 1] scalar (so the output DMA is one descriptor).
Host sums the 8 per-core scalars and divides by B*N.
"""

import sys

import numpy as np

if "/opt/trn_rl_repo" not in sys.path:
    sys.path.insert(0, "/opt/trn_rl_repo")

_B, _N = 32, 512
_NCORES = 8
_BPC = _B // _NCORES            # samples per core
_ROWS = _BPC * _N               # 2048 rows per core
_RPP = _ROWS // 128             # 16 rows per partition
_NT = 1                         # DMA tiles per core
_RPT = _RPP // _NT              # 8 rows per partition per tile


# ---------------------------------------------------------------------------
# Host-side Hungarian matching (same math as the reference, CPU/numpy).
# ---------------------------------------------------------------------------

def _lap(cost):
    """Jonker-Volgenant O(n^3) min-cost assignment on a square float64
    matrix; returns the column assigned to each row."""
    n = cost.shape[0]
    INF = np.inf
    C = np.zeros((n + 1, n + 1), dtype=np.float64)
    C[1:, 1:] = cost
    u = np.zeros(n + 1)
    v = np.zeros(n + 1)
    p = np.zeros(n + 1, dtype=np.int64)
    way = np.zeros(n + 1, dtype=np.int64)
    for i in range(1, n + 1):
        p[0] = i
        j0 = 0
        minv = np.full(n + 1, INF)
        used = np.zeros(n + 1, dtype=bool)
        while True:
            used[j0] = True
            i0 = p[j0]
            free = ~used
            free[0] = False
            cur = C[i0] - u[i0] - v
            upd = free & (cur < minv)
            minv = np.where(upd, cur, minv)
            way[upd] = j0
            mv = np.where(free, minv, INF)
            j1 = int(np.argmin(mv))
            delta = mv[j1]
            u[p[used]] += delta
            v[used] -= delta
            minv = np.where(free, minv - delta, minv)
            j0 = j1
            if p[j0] == 0:
                break
        while j0 != 0:
            j1 = way[j0]
            p[j0] = p[j1]
            j0 = j1
    col_for_row = np.zeros(n, dtype=np.int64)
    col_for_row[p[1:] - 1] = np.arange(n)
    return col_for_row


def _match(D_np):
    """Per-sample min-max normalize (float64) then Hungarian match."""
    B, N, _ = D_np.shape
    cols = np.zeros((B, N), dtype=np.int64)
    try:
        from scipy.optimize import linear_sum_assignment
    except Exception:
        linear_sum_assignment = None
    for b in range(B):
        Db = D_np[b]
        d_min = Db.min()
        d_range = Db.max() - d_min + 1e-8
        C = (Db - d_min) / d_range
        if linear_sum_assignment is not None:
            _, cols[b] = linear_sum_assignment(C)
        else:
            cols[b] = _lap(C)
    return cols


# ---------------------------------------------------------------------------
# Device kernel.
# ---------------------------------------------------------------------------

_NC_CACHE = None
_NC_RAW_CACHE = None


def _build_nc_raw():
    """Raw-Bass (no Tile) variant: explicit semaphores, no tail butterfly."""
    global _NC_RAW_CACHE
    if _NC_RAW_CACHE is not None:
        return _NC_RAW_CACHE

    import concourse.bass as bass
    from concourse import mybir

    nc = bass.Bass(target_bir_lowering=False)

    d_in = nc.dram_tensor("d", [_ROWS, _N], mybir.dt.float32, kind="ExternalInput")
    # last idx column is zeros, used by the warm-up gather
    idx_in = nc.dram_tensor("idx", [128, 5], mybir.dt.uint16,
                            kind="ExternalInput")
    mask_in = nc.dram_tensor("mask", [128, 64], mybir.dt.float32,
                             kind="ExternalInput")
    out = nc.dram_tensor("out", [1, 1], mybir.dt.float32, kind="ExternalOutput")

    # partition p holds rows 16p..16p+15 contiguously -> 32 KiB descriptors
    d_view = d_in.ap().rearrange("(p j) m -> p (j m)", p=128)  # [128, 8192]

    idx_t = nc.alloc_sbuf_tensor("idx_t", [128, 5], mybir.dt.uint16)
    mask_t = nc.alloc_sbuf_tensor("mask_t", [128, 64], mybir.dt.float32)
    gath = nc.alloc_sbuf_tensor("gath", [128, 64], mybir.dt.float32)
    prod = nc.alloc_sbuf_tensor("prod", [128, 64], mybir.dt.float32)
    part = nc.alloc_sbuf_tensor("part", [128, 1], mybir.dt.float32)
    sc = nc.alloc_sbuf_tensor("sc", [1, 1], mybir.dt.float32)
    junk = nc.alloc_sbuf_tensor("junk", [128, 16], mybir.dt.float32)
    wdata = nc.alloc_sbuf_tensor("wdata", [128, 16], mybir.dt.float32)
    data = [nc.alloc_sbuf_tensor(f"data{t}", [128, _RPT * _N], mybir.dt.float32)
            for t in range(_NT)]

    idx_sem = nc.alloc_semaphore("idx_sem")
    mask_sem = nc.alloc_semaphore("mask_sem")
    d_sems = [nc.alloc_semaphore(f"d{t}_sem") for t in range(_NT)]
    out_sem = nc.alloc_semaphore("out_sem")
    gsem = nc.alloc_semaphore("gsem")
    vsem = nc.alloc_semaphore("vsem")
    all_sems = [idx_sem, mask_sem] + d_sems + [out_sem, gsem, vsem]

    with nc.Block() as block:

        @block.sync
        def _(sync):
            for t in range(_NT):
                sync.dma_start(
                    data[t].ap(),
                    d_view[:, t * _RPT * _N:(t + 1) * _RPT * _N],
                ).then_inc(d_sems[t], 16)
            sync.dma_start(idx_t.ap(), idx_in.ap()).then_inc(idx_sem, 16)
            sync.dma_start(mask_t.ap(), mask_in.ap()).then_inc(mask_sem, 16)
            sync.wait_ge(vsem, 3)
            sync.dma_start(out.ap(), sc.ap()).then_inc(out_sem, 16)
            sync.wait_ge(out_sem, 16)

        @block.gpsimd
        def _(gpsimd):
            nc.gpsimd.memset(wdata.ap(), 0.0).then_inc(gsem, 1)
            gpsimd.wait_ge(idx_sem, 16)
            gpsimd.wait_ge(gsem, 1)
            # warm-up: loads the Q7 IndirectCopy ucode while the data DMA
            # is still in flight (zero indices gather junk from wdata)
            nc.gpsimd.indirect_copy(
                junk.ap(),
                wdata.ap(),
                idx_t.ap()[:, 4:5],
                True,
            )
            gpsimd.wait_ge(d_sems[0], 16)
            nc.gpsimd.indirect_copy(
                gath.ap(),
                data[0].ap(),
                idx_t.ap()[:, 0:4],
                True,
            ).then_inc(gsem, 1)
            gpsimd.wait_ge(vsem, 2)
            nc.gpsimd.tensor_reduce(sc.ap(), part.ap(), mybir.AxisListType.C,
                                    mybir.AluOpType.add).then_inc(vsem, 1)

        @block.vector
        def _(vector):
            vector.wait_ge(gsem, 2)
            vector.wait_ge(mask_sem, 16)
            nc.vector.tensor_mul(prod.ap(), gath.ap(), mask_t.ap()).then_inc(vsem, 1)
            vector.wait_ge(vsem, 1)
            nc.vector.tensor_reduce(part.ap(), prod.ap(), mybir.AxisListType.X,
                                    mybir.AluOpType.add).then_inc(vsem, 1)

    # Block exit emitted drain + all-engine barrier; now restore semaphores
    # to 0 so the NEFF can re-execute.
    nums = sorted(s.num for s in all_sems)
    spans = [[nums[0], nums[0]]]
    for n in nums[1:]:
        if n == spans[-1][1] + 1:
            spans[-1][1] = n
        else:
            spans.append([n, n])
    for lo, hi in spans:
        nc.gpsimd.dma_reset(range(lo, hi + 1))
        nc.gpsimd.sem_clear(range(lo, hi + 1))

    _NC_RAW_CACHE = nc
    return nc


def _build_nc():
    global _NC_CACHE
    if _NC_CACHE is not None:
        return _NC_CACHE

    import concourse.bacc as bacc
    import concourse.tile as tile
    from concourse import mybir

    nc = bacc.Bacc("TRN2", target_bir_lowering=False, debug=False)

    d_in = nc.dram_tensor("d", [_ROWS, _N], mybir.dt.float32, kind="ExternalInput")
    idx_in = nc.dram_tensor("idx", [128, _RPP + 1], mybir.dt.uint16,
                            kind="ExternalInput")
    mask_in = nc.dram_tensor("mask", [128, _RPP * 16], mybir.dt.float32,
                             kind="ExternalInput")
    out = nc.dram_tensor("out", [1, 1], mybir.dt.float32, kind="ExternalOutput")

    d_view = d_in.ap().rearrange("(p j) m -> p (j m)", p=128)  # [128, 8192]

    with tile.TileContext(nc) as tc:
        with tc.tile_pool(name="const", bufs=1) as cpool, \
             tc.tile_pool(name="data", bufs=_NT) as dpool:
            idx_t = cpool.tile([128, _RPP + 1], mybir.dt.uint16)
            nc.sync.dma_start(idx_t[:], idx_in.ap())
            mask_t = cpool.tile([128, _RPP * 16], mybir.dt.float32)
            nc.sync.dma_start(mask_t[:], mask_in.ap())

            gath = cpool.tile([128, _RPP * 16], mybir.dt.float32)
            for t in range(_NT):
                dt_ = dpool.tile([128, _RPT * _N], mybir.dt.float32)
                nc.sync.dma_start(
                    dt_[:],
                    d_view[:, t * _RPT * _N:(t + 1) * _RPT * _N],
                )
                nc.gpsimd.indirect_copy(
                    gath[:, t * _RPT * 16:(t + 1) * _RPT * 16],
                    dt_[:],
                    idx_t[:, t * _RPT:(t + 1) * _RPT],
                    True,
                )

            prod = cpool.tile([128, _RPP * 16], mybir.dt.float32)
            part = cpool.tile([128, 1], mybir.dt.float32)
            sc = cpool.tile([1, 1], mybir.dt.float32)
            nc.vector.tensor_mul(prod[:], gath[:], mask_t[:])
            nc.vector.tensor_reduce(part[:], prod[:], mybir.AxisListType.X,
                                    mybir.AluOpType.add)
            nc.gpsimd.tensor_reduce(sc[:], part[:], mybir.AxisListType.C,
                                    mybir.AluOpType.add)
            nc.sync.dma_start(out.ap(), sc[:])

    nc.compile()
    _NC_CACHE = nc
    return nc


def _core_inputs(D_np, cols_flat):
    """Per-core input maps for run_bass_kernel_spmd."""
    # gather slot i is valid for the 4-partition band b = i % 4
    ii = np.arange(64)
    pp = np.arange(128)
    mask = (((pp[:, None] % 16) // 4) == (ii[None, :] % 4)).astype(np.float32)

    in_maps = []
    ii = np.arange(64)
    idx = np.zeros((128, 5), dtype=np.uint16)
    for g in range(8):
        idx[16 * g + ii % 16, ii // 16] = (ii // 4) * _N + (g * 64 + ii)
    for c in range(_NCORES):
        d_shard = D_np[c * _BPC:(c + 1) * _BPC].reshape(_ROWS, _N)
        cshard = cols_flat[c * _ROWS:(c + 1) * _ROWS]
        # rows matched to column v, per sample s
        inv = np.empty((_BPC, _N), dtype=np.int64)
        for s in range(_BPC):
            inv[s, cshard[s * _N:(s + 1) * _N]] = np.arange(_N)
        # quadruple v -> group g=v//64, slot i=v%64, row-slot j=i//4,
        # band b=i%4; sample s sits at partition 16g+4b+s, row-slot j
        v = np.arange(_N)
        g = v // 64
        i = v % 64
        dperm = np.empty((128, _RPP, _N), dtype=np.float32)
        for s in range(_BPC):
            p = 16 * g + 4 * (i % 4) + s
            dperm[p, i // 4, :] = d_shard[s * _N + inv[s, v], :]
        in_maps.append({"d": dperm.reshape(_ROWS, _N), "idx": idx,
                        "mask": mask})
    return in_maps


_USE_RAW = True


def _run_device(D_np, cols_flat, trace=False):
    from concourse.bass_utils import run_bass_kernel_spmd

    nc = _build_nc_raw() if _USE_RAW else _build_nc()
    in_maps = _core_inputs(D_np, cols_flat)
    res = run_bass_kernel_spmd(nc, in_maps, list(range(_NCORES)), trace=trace)
    parts = np.stack([r["out"] for r in res.results])  # [8, 1, 1]
    return parts, res


def kernel(D):
    D_np = np.asarray(D, dtype=np.float32)
    cols = _match(D_np.astype(np.float64))              # [B, N] host, like ref
    cols_flat = cols.reshape(-1).astype(np.int64)
    parts, _ = _run_device(D_np, cols_flat)
    total = parts.astype(np.float64).sum()
    return np.asarray(total / (_B * _N), dtype=np.float32)


# revision 35
# speedup vs baseline: 1.1726x; 1.0492x over previous
"""Trainium2 kernel for nn_NormalizedHungarianLoss.

Semantics (mirrors the reference): per-sample min-max normalize the cost
matrix in float64 on the host, solve the Hungarian assignment on the host
(the reference itself does this on CPU via numpy), then compute the loss
mean(D[b, r, cols[b, r]]) from the ORIGINAL float32 D on device.

Device part (data-parallel over batch, 4 samples per core):
  - The core's [2048, 512] f32 shard is viewed as [128, 8192] (partition p
    holds the 16 consecutive rows 16p..16p+15), giving 16 KiB contiguous
    DMA descriptors per partition; 2 dma_starts of 2 MiB each.
  - One GPSIMD indirect_copy per tile gathers candidate matched elements.
    indirect_copy shares the index list across each 16-partition group
    (out[p, i] = data[p, idx_i]); index i = k*16 + j carries the matched
    element of row 16*(lo+k) + t*8 + j, valid only at partition p = lo+k.
  - DVE multiply by a constant 0/1 mask selecting k == p%16, reduce over
    the free dim -> [128, 1] partials, then a GPSIMD partition-reduce to a
    single [1, 1] scalar (so the output DMA is one descriptor).
Host sums the 8 per-core scalars and divides by B*N.
"""

import sys

import numpy as np

if "/opt/trn_rl_repo" not in sys.path:
    sys.path.insert(0, "/opt/trn_rl_repo")

_B, _N = 32, 512
_NCORES = 8
_BPC = _B // _NCORES            # samples per core
_ROWS = _BPC * _N               # 2048 rows per core
_RPP = _ROWS // 128             # 16 rows per partition
_NT = 1                         # DMA tiles per core
_RPT = _RPP // _NT              # 8 rows per partition per tile


# ---------------------------------------------------------------------------
# Host-side Hungarian matching (same math as the reference, CPU/numpy).
# ---------------------------------------------------------------------------

def _lap(cost):
    """Jonker-Volgenant O(n^3) min-cost assignment on a square float64
    matrix; returns the column assigned to each row."""
    n = cost.shape[0]
    INF = np.inf
    C = np.zeros((n + 1, n + 1), dtype=np.float64)
    C[1:, 1:] = cost
    u = np.zeros(n + 1)
    v = np.zeros(n + 1)
    p = np.zeros(n + 1, dtype=np.int64)
    way = np.zeros(n + 1, dtype=np.int64)
    for i in range(1, n + 1):
        p[0] = i
        j0 = 0
        minv = np.full(n + 1, INF)
        used = np.zeros(n + 1, dtype=bool)
        while True:
            used[j0] = True
            i0 = p[j0]
            free = ~used
            free[0] = False
            cur = C[i0] - u[i0] - v
            upd = free & (cur < minv)
            minv = np.where(upd, cur, minv)
            way[upd] = j0
            mv = np.where(free, minv, INF)
            j1 = int(np.argmin(mv))
            delta = mv[j1]
            u[p[used]] += delta
            v[used] -= delta
            minv = np.where(free, minv - delta, minv)
            j0 = j1
            if p[j0] == 0:
                break
        while j0 != 0:
            j1 = way[j0]
            p[j0] = p[j1]
            j0 = j1
    col_for_row = np.zeros(n, dtype=np.int64)
    col_for_row[p[1:] - 1] = np.arange(n)
    return col_for_row


def _match(D_np):
    """Per-sample min-max normalize (float64) then Hungarian match."""
    B, N, _ = D_np.shape
    cols = np.zeros((B, N), dtype=np.int64)
    try:
        from scipy.optimize import linear_sum_assignment
    except Exception:
        linear_sum_assignment = None
    for b in range(B):
        Db = D_np[b]
        d_min = Db.min()
        d_range = Db.max() - d_min + 1e-8
        C = (Db - d_min) / d_range
        if linear_sum_assignment is not None:
            _, cols[b] = linear_sum_assignment(C)
        else:
            cols[b] = _lap(C)
    return cols


# ---------------------------------------------------------------------------
# Device kernel.
# ---------------------------------------------------------------------------

_NC_CACHE = None
_NC_RAW_CACHE = None


def _build_nc_raw():
    """Raw-Bass (no Tile) variant: explicit semaphores, no tail butterfly."""
    global _NC_RAW_CACHE
    if _NC_RAW_CACHE is not None:
        return _NC_RAW_CACHE

    import concourse.bass as bass
    from concourse import mybir

    nc = bass.Bass(target_bir_lowering=False)

    d_in = nc.dram_tensor("d", [_ROWS, _N], mybir.dt.float32, kind="ExternalInput")
    # last idx column is zeros, used by the warm-up gather
    idx_in = nc.dram_tensor("idx", [128, 5], mybir.dt.uint16,
                            kind="ExternalInput")
    mask_in = nc.dram_tensor("mask", [128, 64], mybir.dt.float32,
                             kind="ExternalInput")
    out = nc.dram_tensor("out", [1, 1], mybir.dt.float32, kind="ExternalOutput")

    # partition p holds rows 16p..16p+15 contiguously -> 32 KiB descriptors
    d_view = d_in.ap().rearrange("(p j) m -> p (j m)", p=128)  # [128, 8192]

    idx_t = nc.alloc_sbuf_tensor("idx_t", [128, 5], mybir.dt.uint16)
    mask_t = nc.alloc_sbuf_tensor("mask_t", [128, 64], mybir.dt.float32)
    gath = nc.alloc_sbuf_tensor("gath", [128, 64], mybir.dt.float32)
    prod = nc.alloc_sbuf_tensor("prod", [128, 64], mybir.dt.float32)
    part = nc.alloc_sbuf_tensor("part", [128, 1], mybir.dt.float32)
    sc = nc.alloc_sbuf_tensor("sc", [1, 1], mybir.dt.float32)
    junk = nc.alloc_sbuf_tensor("junk", [128, 16], mybir.dt.float32)
    wdata = nc.alloc_sbuf_tensor("wdata", [128, 16], mybir.dt.float32)
    data = [nc.alloc_sbuf_tensor(f"data{t}", [128, _RPT * _N], mybir.dt.float32)
            for t in range(_NT)]

    idx_sem = nc.alloc_semaphore("idx_sem")
    mask_sem = nc.alloc_semaphore("mask_sem")
    d_sems = [nc.alloc_semaphore(f"d{t}_sem") for t in range(_NT)]
    out_sem = nc.alloc_semaphore("out_sem")
    gsem = nc.alloc_semaphore("gsem")
    vsem = nc.alloc_semaphore("vsem")
    all_sems = [idx_sem, mask_sem] + d_sems + [out_sem, gsem, vsem]

    with nc.Block() as block:

        @block.sync
        def _(sync):
            sync.dma_start(idx_t.ap(), idx_in.ap()).then_inc(idx_sem, 16)
            for t in range(_NT):
                sync.dma_start(
                    data[t].ap(),
                    d_view[:, t * _RPT * _N:(t + 1) * _RPT * _N],
                ).then_inc(d_sems[t], 16)
            sync.dma_start(mask_t.ap(), mask_in.ap()).then_inc(mask_sem, 16)
            sync.wait_ge(vsem, 3)
            sync.dma_start(out.ap(), sc.ap()).then_inc(out_sem, 16)
            sync.wait_ge(out_sem, 16)

        @block.gpsimd
        def _(gpsimd):
            gpsimd.wait_ge(idx_sem, 16)
            gpsimd.wait_ge(d_sems[0], 16)
            nc.gpsimd.indirect_copy(
                gath.ap(),
                data[0].ap(),
                idx_t.ap()[:, 0:4],
                True,
            ).then_inc(gsem, 1)
            gpsimd.wait_ge(vsem, 2)
            nc.gpsimd.tensor_reduce(sc.ap(), part.ap(), mybir.AxisListType.C,
                                    mybir.AluOpType.add).then_inc(vsem, 1)

        @block.vector
        def _(vector):
            vector.wait_ge(gsem, 1)
            vector.wait_ge(mask_sem, 16)
            nc.vector.tensor_mul(prod.ap(), gath.ap(), mask_t.ap()).then_inc(vsem, 1)
            vector.wait_ge(vsem, 1)
            nc.vector.tensor_reduce(part.ap(), prod.ap(), mybir.AxisListType.X,
                                    mybir.AluOpType.add).then_inc(vsem, 1)

    # Block exit emitted drain + all-engine barrier; now restore semaphores
    # to 0 so the NEFF can re-execute.
    nums = sorted(s.num for s in all_sems)
    spans = [[nums[0], nums[0]]]
    for n in nums[1:]:
        if n == spans[-1][1] + 1:
            spans[-1][1] = n
        else:
            spans.append([n, n])
    for lo, hi in spans:
        nc.gpsimd.dma_reset(range(lo, hi + 1))
        nc.gpsimd.sem_clear(range(lo, hi + 1))

    _NC_RAW_CACHE = nc
    return nc


def _build_nc():
    global _NC_CACHE
    if _NC_CACHE is not None:
        return _NC_CACHE

    import concourse.bacc as bacc
    import concourse.tile as tile
    from concourse import mybir

    nc = bacc.Bacc("TRN2", target_bir_lowering=False, debug=False)

    d_in = nc.dram_tensor("d", [_ROWS, _N], mybir.dt.float32, kind="ExternalInput")
    idx_in = nc.dram_tensor("idx", [128, _RPP + 1], mybir.dt.uint16,
                            kind="ExternalInput")
    mask_in = nc.dram_tensor("mask", [128, _RPP * 16], mybir.dt.float32,
                             kind="ExternalInput")
    out = nc.dram_tensor("out", [1, 1], mybir.dt.float32, kind="ExternalOutput")

    d_view = d_in.ap().rearrange("(p j) m -> p (j m)", p=128)  # [128, 8192]

    with tile.TileContext(nc) as tc:
        with tc.tile_pool(name="const", bufs=1) as cpool, \
             tc.tile_pool(name="data", bufs=_NT) as dpool:
            idx_t = cpool.tile([128, _RPP + 1], mybir.dt.uint16)
            nc.sync.dma_start(idx_t[:], idx_in.ap())
            mask_t = cpool.tile([128, _RPP * 16], mybir.dt.float32)
            nc.sync.dma_start(mask_t[:], mask_in.ap())

            gath = cpool.tile([128, _RPP * 16], mybir.dt.float32)
            for t in range(_NT):
                dt_ = dpool.tile([128, _RPT * _N], mybir.dt.float32)
                nc.sync.dma_start(
                    dt_[:],
                    d_view[:, t * _RPT * _N:(t + 1) * _RPT * _N],
                )
                nc.gpsimd.indirect_copy(
                    gath[:, t * _RPT * 16:(t + 1) * _RPT * 16],
                    dt_[:],
                    idx_t[:, t * _RPT:(t + 1) * _RPT],
                    True,
                )

            prod = cpool.tile([128, _RPP * 16], mybir.dt.float32)
            part = cpool.tile([128, 1], mybir.dt.float32)
            sc = cpool.tile([1, 1], mybir.dt.float32)
            nc.vector.tensor_mul(prod[:], gath[:], mask_t[:])
            nc.vector.tensor_reduce(part[:], prod[:], mybir.AxisListType.X,
                                    mybir.AluOpType.add)
            nc.gpsimd.tensor_reduce(sc[:], part[:], mybir.AxisListType.C,
                                    mybir.AluOpType.add)
            nc.sync.dma_start(out.ap(), sc[:])

    nc.compile()
    _NC_CACHE = nc
    return nc


def _core_inputs(D_np, cols_flat):
    """Per-core input maps for run_bass_kernel_spmd."""
    # gather slot i is valid for the 4-partition band b = i % 4
    ii = np.arange(64)
    pp = np.arange(128)
    mask = (((pp[:, None] % 16) // 4) == (ii[None, :] % 4)).astype(np.float32)

    in_maps = []
    ii = np.arange(64)
    idx = np.zeros((128, 5), dtype=np.uint16)
    for g in range(8):
        idx[16 * g + ii % 16, ii // 16] = (ii // 4) * _N + (g * 64 + ii)
    for c in range(_NCORES):
        d_shard = D_np[c * _BPC:(c + 1) * _BPC].reshape(_ROWS, _N)
        cshard = cols_flat[c * _ROWS:(c + 1) * _ROWS]
        # rows matched to column v, per sample s
        inv = np.empty((_BPC, _N), dtype=np.int64)
        for s in range(_BPC):
            inv[s, cshard[s * _N:(s + 1) * _N]] = np.arange(_N)
        # quadruple v -> group g=v//64, slot i=v%64, row-slot j=i//4,
        # band b=i%4; sample s sits at partition 16g+4b+s, row-slot j
        v = np.arange(_N)
        g = v // 64
        i = v % 64
        dperm = np.empty((128, _RPP, _N), dtype=np.float32)
        for s in range(_BPC):
            p = 16 * g + 4 * (i % 4) + s
            dperm[p, i // 4, :] = d_shard[s * _N + inv[s, v], :]
        in_maps.append({"d": dperm.reshape(_ROWS, _N), "idx": idx,
                        "mask": mask})
    return in_maps


_USE_RAW = True


def _run_device(D_np, cols_flat, trace=False):
    from concourse.bass_utils import run_bass_kernel_spmd

    nc = _build_nc_raw() if _USE_RAW else _build_nc()
    in_maps = _core_inputs(D_np, cols_flat)
    res = run_bass_kernel_spmd(nc, in_maps, list(range(_NCORES)), trace=trace)
    parts = np.stack([r["out"] for r in res.results])  # [8, 1, 1]
    return parts, res


def kernel(D):
    D_np = np.asarray(D, dtype=np.float32)
    cols = _match(D_np.astype(np.float64))              # [B, N] host, like ref
    cols_flat = cols.reshape(-1).astype(np.int64)
    parts, _ = _run_device(D_np, cols_flat)
    total = parts.astype(np.float64).sum()
    return np.asarray(total / (_B * _N), dtype=np.float32)
